# revision 1
# baseline (speedup 1.0000x reference)
"""Multi-head attention TRN2 kernel (B=4, S=2048, E=128, H=8) on 8 NeuronCores.

Sharding: core c handles batch b = c // 2 and head group g = c % 2
(heads 4g .. 4g+3).  Each core computes the partial output
outT_partial[e_out, s] = sum_{h in group} (softmax(QK^T/sqrt(E)) V)_h @ Wo_h
for its batch, transposed.  Host sums the two head-group partials per batch,
transposes, and adds bo.

Device algorithm (all-transposed layout, no attention transposes needed):
  qT   [e, s]        via PE transpose of q
  QT_h = Wq_h^T qT   [f, s]  (lhsT = Wq natural layout)
  KT_h likewise      [f, t]
  V_h  = (qT-block)^T Wv_h   [t, f] per 128-block of t
  scoresT[t, s] = KT_h-block^T @ QT  -> exp on ScalarE -> attnT (bf16)
  denom[s] = ones^T @ (DVE-folded attnT)   (column sums)
  ZT[f, s] = sum_t V-block^T... accumulated over t blocks in PSUM
  ZT_norm = ZT * (1/denom);  outT += Wo_h^T @ ZT_norm
"""

import sys

for _p in ("/opt/trn_rl_repo",):
    if _p not in sys.path:
        sys.path.insert(0, _p)

import numpy as np

import concourse.bass as bass
import concourse.mybir as mybir
import concourse.tile as tile
from concourse.bass_utils import run_bass_kernel_spmd
from concourse.masks import make_identity

F32 = mybir.dt.float32
F32R = mybir.dt.float32r
BF16 = mybir.dt.bfloat16
F16 = mybir.dt.float16

B, S, E, H = 4, 2048, 128, 8
NH = 4          # heads per core
TB = S // 128   # 16 t blocks
S_SPLIT = 2     # s-direction split per head (pipelining unit)
SW = S // S_SPLIT        # 1024
NC_CHUNK = 512           # psum-bank chunk
SCALE = 1.0 / np.sqrt(E)

_prog_cache = {}


def build_program():
    if "nc" in _prog_cache:
        return _prog_cache["nc"]

    import concourse.bacc as bacc

    nc = bacc.Bacc("TRN2", target_bir_lowering=False, debug=False)

    q_d = nc.dram_tensor("q", [S, E], F32, kind="ExternalInput").ap()
    wq_d = nc.dram_tensor("Wq", [NH, E, E], F32, kind="ExternalInput").ap()
    wk_d = nc.dram_tensor("Wk", [NH, E, E], F32, kind="ExternalInput").ap()
    wv_d = nc.dram_tensor("Wv", [NH, E, E], F32, kind="ExternalInput").ap()
    wo_d = nc.dram_tensor("Wo", [NH, E, E], F32, kind="ExternalInput").ap()
    bq_d = nc.dram_tensor("bq", [NH, E], F32, kind="ExternalInput").ap()
    bk_d = nc.dram_tensor("bk", [NH, E], F32, kind="ExternalInput").ap()
    out_d = nc.dram_tensor("out", [E, S], F32, kind="ExternalOutput").ap()

    with tile.TileContext(nc) as tc:
        _emit(nc, tc, q_d, wq_d, wk_d, wv_d, wo_d, bq_d, bk_d, out_d)

    nc.compile()
    _prog_cache["nc"] = nc
    return nc


def _emit(nc, tc, q_d, wq_d, wk_d, wv_d, wo_d, bq_d, bk_d, out_d):
    from contextlib import ExitStack

    ctx = ExitStack()
    consts = ctx.enter_context(tc.tile_pool(name="consts", bufs=1))
    heads = ctx.enter_context(tc.tile_pool(name="heads", bufs=2))
    attns = ctx.enter_context(tc.tile_pool(name="attns", bufs=2))
    folds = ctx.enter_context(tc.tile_pool(name="folds", bufs=1))
    works = ctx.enter_context(tc.tile_pool(name="works", bufs=2))
    psum_big = ctx.enter_context(tc.tile_pool(name="psum_big", bufs=2, space="PSUM"))
    psum_work = ctx.enter_context(tc.tile_pool(name="psum_work", bufs=2, space="PSUM"))

    # ---- constants / preload ----
    ident = consts.tile([128, 128], F32, tag="ident")
    make_identity(nc, ident)
    ones_bf = consts.tile([128, 128], F16, tag="ones")
    nc.vector.memset(ones_bf, 1.0)

    # q first: it heads the critical path (transposes -> proj -> scores)
    q_sb = consts.tile([128, TB, 128], F32, tag="qsb")  # [s_sub, s_blk, e]
    q_r = q_d.rearrange("(sb p) e -> p sb e", p=128)
    for qc in range(4):
        nc.sync.dma_start(
            out=q_sb[:, qc * (TB // 4) : (qc + 1) * (TB // 4), :],
            in_=q_r[:, qc * (TB // 4) : (qc + 1) * (TB // 4), :],
        )

    w_stage = consts.tile([128, 4, NH, 128], F32, tag="wstage")
    for i, wd in enumerate((wq_d, wk_d, wv_d, wo_d)):
        nc.sync.dma_start(out=w_stage[:, i], in_=wd.rearrange("h i j -> i h j"))
    wq_sb = consts.tile([128, NH, 128], F16, tag="wq")  # [e_in, h, e_out]
    nc.vector.tensor_copy(wq_sb, w_stage[:, 0])
    wk_sb = consts.tile([128, NH, 128], F16, tag="wk")
    nc.vector.tensor_copy(wk_sb, w_stage[:, 1])
    wv_sb = consts.tile([128, NH, 128], F16, tag="wv")
    nc.vector.tensor_copy(wv_sb, w_stage[:, 2])
    wo_sb = consts.tile([128, NH, 128], F16, tag="wo")  # [f, h, g]
    nc.vector.tensor_copy(wo_sb, w_stage[:, 3])

    bq_sb = consts.tile([128, NH], F32, tag="bq")  # [f, h]
    nc.sync.dma_start(out=bq_sb, in_=bq_d.rearrange("h f -> f h"))
    bk_sb = consts.tile([128, NH], F32, tag="bk")
    nc.sync.dma_start(out=bk_sb, in_=bk_d.rearrange("h f -> f h"))

    # ---- qT via PE transposes ----
    qT = consts.tile([128, TB, 128], F16, tag="qT")  # [e, s_blk, s_sub]
    for sb in range(TB):
        pt = psum_work.tile([128, 128], F32, tag="work")
        nc.tensor.transpose(pt, q_sb[:, sb, :], ident)
        nc.vector.tensor_copy(qT[:, sb, :], pt)
    qT_flat = qT.rearrange("e sb p -> e (sb p)")  # [e, s]

    # accumulators for output (ping-pong per s-half)
    acc_a = [
        consts.tile([128, SW], F32, tag=f"acc_a{sh}", name=f"acc_a{sh}")
        for sh in range(S_SPLIT)
    ]
    acc_b = [
        consts.tile([128, SW], F32, tag=f"acc_b{sh}", name=f"acc_b{sh}")
        for sh in range(S_SPLIT)
    ]

    def emit_proj(h):
        # ---- projections for head h ----
        qt_h = heads.tile([128, S], F16, tag="QT", name=f"qt_{h}")  # [f, s]
        kt_h = heads.tile([128, S], F16, tag="KT", name=f"kt_{h}")  # [f, t]
        for j in range(S // NC_CHUNK):
            ps = psum_work.tile([128, NC_CHUNK], F32, tag="work", name=f"qtp_{h}_{j}")
            nc.tensor.matmul(
                ps,
                lhsT=wq_sb[:, h, :],
                rhs=qT_flat[:, j * NC_CHUNK : (j + 1) * NC_CHUNK],
                start=True,
                stop=True,
            )
            nc.vector.tensor_scalar_add(
                qt_h[:, j * NC_CHUNK : (j + 1) * NC_CHUNK], ps, bq_sb[:, h : h + 1]
            )
        for j in range(S // NC_CHUNK):
            ps = psum_work.tile([128, NC_CHUNK], F32, tag="work", name=f"ktp_{h}_{j}")
            nc.tensor.matmul(
                ps,
                lhsT=wk_sb[:, h, :],
                rhs=qT_flat[:, j * NC_CHUNK : (j + 1) * NC_CHUNK],
                start=True,
                stop=True,
            )
            nc.vector.tensor_scalar_add(
                kt_h[:, j * NC_CHUNK : (j + 1) * NC_CHUNK], ps, bk_sb[:, h : h + 1]
            )
        return qt_h, kt_h

    proj = {0: emit_proj(0)}

    # ---- V projection: all heads at once, one N=512 matmul per t-block ----
    wv_all = wv_sb.rearrange("e h f -> e (h f)")  # [e, 512]
    v_all = consts.tile([128, TB, NH, 128], F16, tag="vall")  # [t_sub, tb, h, f]
    for tb in range(TB):
        vps = psum_work.tile([128, NC_CHUNK], F32, tag="work", name=f"vp_{tb}")
        nc.tensor.matmul(vps, lhsT=qT[:, tb, :], rhs=wv_all, start=True, stop=True)
        nc.vector.tensor_copy(
            v_all.rearrange("p t h f -> p (t h f)")[:, tb * 512 : (tb + 1) * 512], vps
        )

    for h in range(NH):
        qt_h, kt_h = proj.pop(h)
        for sh in range(S_SPLIT):
            if sh == 1 and h + 1 < NH:
                proj[h + 1] = emit_proj(h + 1)
            s0 = sh * SW
            # ---- scores + exp + fold + AV, interleaved per t-block ----
            attnT = attns.tile([128, TB, SW], F16, tag="attnT")  # [t_sub, t_blk, s]
            f1 = folds.tile([128, TB // 2, SW], F16, tag="f1")
            f2 = folds.tile([128, TB // 4, SW], F16, tag="f2")
            zts = [
                psum_work.tile([128, NC_CHUNK], F32, tag="zt", name=f"zt_{h}_{sh}_{c}")
                for c in range(SW // NC_CHUNK)
            ]
            dns = [
                psum_work.tile([128, NC_CHUNK], F32, tag="work", name=f"dn_{h}_{sh}_{c}")
                for c in range(SW // NC_CHUNK)
            ]
            for tb in range(TB):
                sc = psum_big.tile([128, SW], F32, tag="sc")
                for j in range(SW // NC_CHUNK):
                    nc.tensor.matmul(
                        sc[:, j * NC_CHUNK : (j + 1) * NC_CHUNK],
                        lhsT=kt_h[:, tb * 128 : (tb + 1) * 128],
                        rhs=qt_h[
                            :, s0 + j * NC_CHUNK : s0 + (j + 1) * NC_CHUNK
                        ],
                        start=True,
                        stop=True,
                    )
                nc.scalar.activation(
                    attnT[:, tb, :], sc, mybir.ActivationFunctionType.Exp, scale=SCALE
                )
                for c in range(SW // NC_CHUNK):
                    nc.tensor.matmul(
                        zts[c],
                        lhsT=v_all[:, tb, h, :],
                        rhs=attnT[:, tb, c * NC_CHUNK : (c + 1) * NC_CHUNK],
                        start=(tb == 0),
                        stop=(tb == TB - 1),
                    )
                if tb >= 8:
                    i = tb - 8
                    nc.vector.tensor_add(
                        f1[:, i, :], attnT[:, i, :], attnT[:, tb, :]
                    )
                if tb >= 12:
                    i = tb - 12
                    nc.vector.tensor_add(f2[:, i, :], f1[:, i, :], f1[:, i + 4, :])
                    # denominator partial accumulation per chunk
                    for c in range(SW // NC_CHUNK):
                        nc.tensor.matmul(
                            dns[c],
                            lhsT=ones_bf,
                            rhs=f2[:, i, c * NC_CHUNK : (c + 1) * NC_CHUNK],
                            start=(i == 0),
                            stop=(i == TB // 4 - 1),
                        )

            for c in range(SW // NC_CHUNK):
                c0 = c * NC_CHUNK
                recip = works.tile([128, NC_CHUNK], F32, tag="recip")
                nc.vector.reciprocal_approx_fast(recip, dns[c])
                ztn = works.tile([128, NC_CHUNK], F16, tag="ztn")
                nc.vector.tensor_mul(ztn, zts[c], recip)
                # ---- output projection ----
                wo_ps = psum_work.tile([128, NC_CHUNK], F32, tag="work")
                nc.tensor.matmul(
                    wo_ps,
                    lhsT=wo_sb[:, h, :],
                    rhs=ztn,
                    start=True,
                    stop=True,
                )
                csl = slice(s0 + c0, s0 + c0 + NC_CHUNK)
                asl = slice(c0, c0 + NC_CHUNK)
                if h == 0:
                    nc.vector.tensor_copy(acc_a[sh][:, asl], wo_ps)
                elif h == 1:
                    nc.vector.tensor_add(acc_b[sh][:, asl], acc_a[sh][:, asl], wo_ps)
                elif h == 2:
                    nc.vector.tensor_add(acc_a[sh][:, asl], acc_b[sh][:, asl], wo_ps)
                else:
                    osb = works.tile([128, NC_CHUNK], F32, tag="osb")
                    nc.vector.tensor_add(osb, acc_a[sh][:, asl], wo_ps)
                    nc.sync.dma_start(out=out_d[:, csl], in_=osb)

    ctx.close()


def _in_maps(inputs):
    q = np.asarray(inputs["q"], dtype=np.float32)
    Wq = np.asarray(inputs["Wq"], dtype=np.float32)
    bq = np.asarray(inputs["bq"], dtype=np.float32)
    Wk = np.asarray(inputs["Wk"], dtype=np.float32)
    bk = np.asarray(inputs["bk"], dtype=np.float32)
    Wv = np.asarray(inputs["Wv"], dtype=np.float32)
    bv = np.asarray(inputs["bv"], dtype=np.float32)
    Wo = np.asarray(inputs["Wo"], dtype=np.float32).reshape(H, E, E)
    maps = []
    for c in range(8):
        b = c // 2
        hs = slice(4 * (c % 2), 4 * (c % 2) + 4)
        maps.append(
            {
                "q": np.ascontiguousarray(q[b]),
                "Wq": np.ascontiguousarray(Wq[hs]),
                "Wk": np.ascontiguousarray(Wk[hs]),
                "Wv": np.ascontiguousarray(Wv[hs]),
                "Wo": np.ascontiguousarray(Wo[hs]),
                "bq": np.ascontiguousarray(bq[hs]),
                "bk": np.ascontiguousarray(bk[hs]),
            }
        )
    return maps


def kernel(**inputs):
    nc = build_program()
    maps = _in_maps(inputs)
    res = run_bass_kernel_spmd(nc, maps, core_ids=list(range(8)))
    bo = np.asarray(inputs["bo"], dtype=np.float32)
    bv = np.asarray(inputs["bv"], dtype=np.float32)
    Wo = np.asarray(inputs["Wo"], dtype=np.float32).reshape(H, E, E)
    # V-bias contribution folded out of the device kernel:
    # sum_h softmax(..)@ (qWv + bv) @ Wo_h = device_partials + sum_h bv_h @ Wo_h
    bo_eff = bo + np.einsum("he,hef->f", bv, Wo).astype(np.float32)
    out = np.empty((B, S, E), dtype=np.float32)
    for b in range(B):
        part = res.results[2 * b]["out"] + res.results[2 * b + 1]["out"]
        out[b] = part.T + bo_eff
    return out



# revision 5
# speedup vs baseline: 1.0547x; 1.0547x over previous
"""Multi-head attention TRN2 kernel (B=4, S=2048, E=128, H=8) on 8 NeuronCores.

Sharding: core c handles batch b = c // 2 and head group g = c % 2
(heads 4g .. 4g+3).  Each core computes the partial output
outT_partial[e_out, s] = sum_{h in group} (softmax(QK^T/sqrt(E)) V)_h @ Wo_h
for its batch, transposed.  Host sums the two head-group partials per batch,
transposes, and adds bo (plus the host-folded bv and bk contributions).

v2 design (vs v1 baseline at ~204 us):
  - host sends q pre-transposed as f16 [E, S]: no PE transposes, no qT casts
  - bk dropped entirely (adds a per-query constant to scores -> cancels in
    softmax); bv folded into the host-side output bias (as before)
  - attn weights (exp of scores) written by ScalarE directly as fp8e4;
    V projection cast to fp8e4 -> AV matmul and the softmax-denominator
    matmul both run in DoubleRow mode (contract 2 t-blocks of 128 per
    matmul), halving PE time for those stages
  - denominator computed as a tail burst of 8 DoubleRow matmuls with an
    fp8 ones matrix (no DVE fold tree at all)
  - PSUM: sc [128,1024] x2bufs (4 banks) + zts 2 banks + 2 rotating work
    banks (proj / V / dns / outproj)
  - next head's Q/K projections and the V projection are drip-fed into the
    tb loop (2 ops per iteration) so PE work overlaps the exp stream
"""

import sys

for _p in ("/opt/trn_rl_repo",):
    if _p not in sys.path:
        sys.path.insert(0, _p)

import numpy as np

import concourse.bass as bass
import concourse.mybir as mybir
import concourse.tile as tile
from concourse.bass_utils import run_bass_kernel_spmd

F32 = mybir.dt.float32
F16 = mybir.dt.float16
F8 = mybir.dt.float8e4
DR = mybir.MatmulPerfMode.DoubleRow
EXP = mybir.ActivationFunctionType.Exp

B, S, E, H = 4, 2048, 128, 8
NH = 4          # heads per core
TB = S // 128   # 16 t blocks
SW = 1024       # s-half width
NC = 512        # psum-bank chunk
SCALE = 1.0 / np.sqrt(E)

_prog_cache = {}


def build_program():
    if "nc" in _prog_cache:
        return _prog_cache["nc"]

    import concourse.bacc as bacc

    nc = bacc.Bacc("TRN2", target_bir_lowering=False, debug=False)

    qt_d = nc.dram_tensor("qT", [E, S], F16, kind="ExternalInput").ap()
    wq_d = nc.dram_tensor("Wq", [E, NH, E], F16, kind="ExternalInput").ap()
    wk_d = nc.dram_tensor("Wk", [E, NH, E], F16, kind="ExternalInput").ap()
    wv_d = nc.dram_tensor("Wv", [E, NH, E], F16, kind="ExternalInput").ap()
    wo_d = nc.dram_tensor("Wo", [E, NH, E], F16, kind="ExternalInput").ap()
    bq_d = nc.dram_tensor("bq", [E, NH], F32, kind="ExternalInput").ap()
    out_d = nc.dram_tensor("out", [E, S], F32, kind="ExternalOutput").ap()

    with tile.TileContext(nc) as tc:
        _emit(nc, tc, qt_d, wq_d, wk_d, wv_d, wo_d, bq_d, out_d)

    nc.compile()
    _prog_cache["nc"] = nc
    return nc


def _emit(nc, tc, qt_d, wq_d, wk_d, wv_d, wo_d, bq_d, out_d):
    from collections import deque
    from contextlib import ExitStack

    ctx = ExitStack()
    consts = ctx.enter_context(tc.tile_pool(name="consts", bufs=1))
    heads = ctx.enter_context(tc.tile_pool(name="heads", bufs=2))
    attns = ctx.enter_context(tc.tile_pool(name="attns", bufs=2))
    works = ctx.enter_context(tc.tile_pool(name="works", bufs=2))
    psum_sc = ctx.enter_context(tc.tile_pool(name="psum_sc", bufs=2, space="PSUM"))
    psum_av = ctx.enter_context(tc.tile_pool(name="psum_av", bufs=2, space="PSUM"))
    psum_wk = ctx.enter_context(tc.tile_pool(name="psum_wk", bufs=2, space="PSUM"))

    # ---- constants / inputs ----
    qT = consts.tile([128, S], F16, tag="qT")  # [e, s]
    for j in range(4):
        nc.sync.dma_start(out=qT[:, j * 512 : (j + 1) * 512],
                          in_=qt_d[:, j * 512 : (j + 1) * 512])
    wq = consts.tile([128, NH, 128], F16, tag="wq")  # [e_in, h, e_out]
    nc.sync.dma_start(out=wq, in_=wq_d)
    wk = consts.tile([128, NH, 128], F16, tag="wk")
    nc.sync.dma_start(out=wk, in_=wk_d)
    wv = consts.tile([128, NH, 128], F16, tag="wv")
    nc.sync.dma_start(out=wv, in_=wv_d)
    wo = consts.tile([128, NH, 128], F16, tag="wo")  # [f, h, g]
    nc.sync.dma_start(out=wo, in_=wo_d)
    bq = consts.tile([128, NH], F32, tag="bq")  # [f, h]
    nc.sync.dma_start(out=bq, in_=bq_d)

    ones8 = consts.tile([128, 2, 128], F8, tag="ones8")
    nc.vector.memset(ones8, 1.0)

    v8 = consts.tile([128, TB, NH * 128], F8, tag="v8")  # [t_sub, tb, (h f)]
    wv_all = wv.rearrange("e h f -> e (h f)")

    acc_a = [
        consts.tile([128, SW], F32, tag=f"acca{s}", name=f"acca{s}") for s in range(2)
    ]
    acc_b = [
        consts.tile([128, SW], F32, tag=f"accb{s}", name=f"accb{s}") for s in range(2)
    ]

    def v_block(tb):
        ps = psum_wk.tile([128, NC], F32, tag="work", name=f"vps_{tb}")
        nc.tensor.matmul(ps, lhsT=qT[:, tb * 128 : (tb + 1) * 128], rhs=wv_all,
                         start=True, stop=True)
        nc.vector.tensor_copy(v8[:, tb, :], ps)

    def proj_q(h, qt_h, j):
        ps = psum_wk.tile([128, NC], F32, tag="work", name=f"qp{h}_{j}")
        nc.tensor.matmul(ps, lhsT=wq[:, h, :], rhs=qT[:, j * 512 : (j + 1) * 512],
                         start=True, stop=True)
        nc.vector.tensor_scalar_add(qt_h[:, j * 512 : (j + 1) * 512], ps,
                                    bq[:, h : h + 1])

    def proj_k(h, kt_h, j):
        ps = psum_wk.tile([128, NC], F32, tag="work", name=f"kp{h}_{j}")
        nc.tensor.matmul(ps, lhsT=wk[:, h, :], rhs=qT[:, j * 512 : (j + 1) * 512],
                         start=True, stop=True)
        nc.vector.tensor_copy(kt_h[:, j * 512 : (j + 1) * 512], ps)

    def alloc_head(h):
        qt_h = heads.tile([128, S], F16, tag="QT", name=f"qt{h}")  # [f, s]
        kt_h = heads.tile([128, S], F16, tag="KT", name=f"kt{h}")  # [f, t]
        return qt_h, kt_h

    # prologue: only what the first scores matmul needs, then drip the rest
    cur = alloc_head(0)
    proj_q(0, cur[0], 0)
    proj_q(0, cur[0], 1)
    proj_k(0, cur[1], 0)
    v_block(0)
    v_block(1)

    nxt = None
    for h in range(NH):
        qt_h, kt_h = cur
        for sh in range(2):
            extras = deque()
            if h == 0 and sh == 0:
                extras += [lambda j=j: proj_k(0, kt_h, j) for j in (1, 2, 3)]
                extras += [lambda j=j: proj_q(0, qt_h, j) for j in (2, 3)]
                extras += [lambda t=t: v_block(t) for t in range(2, TB)]
            if sh == 1 and h + 1 < NH:
                nxt = alloc_head(h + 1)
                hh, nq, nk = h + 1, nxt[0], nxt[1]
                extras += [lambda j=j: proj_q(hh, nq, j) for j in range(4)]
                extras += [lambda j=j: proj_k(hh, nk, j) for j in range(4)]

            s0 = sh * SW
            attnT = attns.tile([128, TB, SW], F8, tag="attnT", name=f"at{h}{sh}")
            zts = [
                psum_av.tile([128, NC], F32, tag="zt", name=f"zt{h}{sh}{c}")
                for c in range(2)
            ]
            for tb in range(TB):
                for _ in range(2):
                    if extras:
                        extras.popleft()()
                sc = psum_sc.tile([128, SW], F32, tag="sc", name=f"sc{h}{sh}{tb}")
                for c in range(2):
                    nc.tensor.matmul(
                        sc[:, c * 512 : (c + 1) * 512],
                        lhsT=kt_h[:, tb * 128 : (tb + 1) * 128],
                        rhs=qt_h[:, s0 + c * 512 : s0 + (c + 1) * 512],
                        start=True, stop=True,
                    )
                nc.scalar.activation(attnT[:, tb, :], sc, EXP, scale=SCALE)
                if tb % 2 == 1:
                    p = tb // 2
                    for c in range(2):
                        nc.tensor.matmul(
                            zts[c],
                            lhsT=v8[:, tb - 1 : tb + 1, h * 128 : (h + 1) * 128],
                            rhs=attnT[:, tb - 1 : tb + 1, c * 512 : (c + 1) * 512],
                            start=(p == 0), stop=(p == 7), perf_mode=DR,
                        )
            while extras:
                extras.popleft()()

            # ---- tail: denominator burst + normalize + output projection ----
            for c in range(2):
                dn = psum_wk.tile([128, NC], F32, tag="work", name=f"dn{h}{sh}{c}")
                for p in range(8):
                    nc.tensor.matmul(
                        dn,
                        lhsT=ones8,
                        rhs=attnT[:, 2 * p : 2 * p + 2, c * 512 : (c + 1) * 512],
                        start=(p == 0), stop=(p == 7), perf_mode=DR,
                    )
                recip = works.tile([128, NC], F32, tag="recip")
                nc.vector.reciprocal_approx_fast(recip, dn)
                ztn = works.tile([128, NC], F16, tag="ztn")
                nc.vector.tensor_mul(ztn, zts[c], recip)
                wo_ps = psum_wk.tile([128, NC], F32, tag="work", name=f"wop{h}{sh}{c}")
                nc.tensor.matmul(wo_ps, lhsT=wo[:, h, :], rhs=ztn,
                                 start=True, stop=True)
                asl = slice(c * 512, (c + 1) * 512)
                csl = slice(s0 + c * 512, s0 + (c + 1) * 512)
                if h == 0:
                    nc.vector.tensor_copy(acc_a[sh][:, asl], wo_ps)
                elif h == 1:
                    nc.vector.tensor_add(acc_b[sh][:, asl], acc_a[sh][:, asl], wo_ps)
                elif h == 2:
                    nc.vector.tensor_add(acc_a[sh][:, asl], acc_b[sh][:, asl], wo_ps)
                else:
                    osb = works.tile([128, NC], F32, tag="osb")
                    nc.vector.tensor_add(osb, acc_a[sh][:, asl], wo_ps)
                    nc.sync.dma_start(out=out_d[:, csl], in_=osb)
        if h + 1 < NH:
            cur = nxt

    ctx.close()


def _in_maps(inputs):
    q = np.asarray(inputs["q"], dtype=np.float32)
    Wq = np.asarray(inputs["Wq"], dtype=np.float32)
    bq = np.asarray(inputs["bq"], dtype=np.float32)
    Wk = np.asarray(inputs["Wk"], dtype=np.float32)
    Wv = np.asarray(inputs["Wv"], dtype=np.float32)
    Wo = np.asarray(inputs["Wo"], dtype=np.float32).reshape(H, E, E)

    def warr(w, hs):  # [h, e_in, e_out] slice -> [e_in, h, e_out] f16
        return np.ascontiguousarray(w[hs].transpose(1, 0, 2)).astype(np.float16)

    maps = []
    for c in range(8):
        b = c // 2
        hs = slice(4 * (c % 2), 4 * (c % 2) + 4)
        maps.append(
            {
                "qT": np.ascontiguousarray(q[b].T).astype(np.float16),
                "Wq": warr(Wq, hs),
                "Wk": warr(Wk, hs),
                "Wv": warr(Wv, hs),
                "Wo": warr(Wo, hs),
                "bq": np.ascontiguousarray(bq[hs].T),
            }
        )
    return maps


def kernel(**inputs):
    nc = build_program()
    maps = _in_maps(inputs)
    res = run_bass_kernel_spmd(nc, maps, core_ids=list(range(8)))
    bo = np.asarray(inputs["bo"], dtype=np.float32)
    bv = np.asarray(inputs["bv"], dtype=np.float32)
    Wo = np.asarray(inputs["Wo"], dtype=np.float32).reshape(H, E, E)
    # V-bias contribution folded out of the device kernel:
    # sum_h softmax(..)@ (qWv + bv) @ Wo_h = device_partials + sum_h bv_h @ Wo_h
    bo_eff = bo + np.einsum("he,hef->f", bv, Wo).astype(np.float32)
    out = np.empty((B, S, E), dtype=np.float32)
    for b in range(B):
        part = res.results[2 * b]["out"] + res.results[2 * b + 1]["out"]
        out[b] = part.T + bo_eff
    return out
